# revision 1
# baseline (speedup 1.0000x reference)
"""BoundaryLoss kernel for Trainium2 (8 NeuronCores, data-parallel over batch).

Problem: for each (batch, waypoint), find the nearest boundary point (argmin
over N=4096 of euclidean distance), take dot(waypoint - closest_pt,
closest_normal), apply exp_relu, and mean over everything.

Per core (4 of the 32 batches; per batch 2 chunks of 128 waypoints):
  - PE: score[w, n] = sum_d wp[w,d]*bp[n,d] - 0.5*||bp[n]||^2
    = -0.5*sq_dist + const(w), so argmax_n score == argmin_n dist.
    K=6 fp32 matmuls (512 cols / PSUM bank); two [128, 2048] halves per tile.
  - ACT copies half 0 PSUM->SBUF; DVE folds: f = max(h0, h1) elementwise
    (one 2048-cycle pass consuming all 4096 scores), then max8 + max_index
    on the folded half (first-occurrence => reference tie-break).
  - The fold leaves two candidates (j, j+2048). A host-packed table row j
    holds bp/normal for BOTH, gathered with one indirect DMA per tile.
    Batched DVE ops recompute both squared distances exactly, pick the
    winner (<= prefers the lower index), and form dot(delta, normal).
  - exp_relu + row-sum; host sums the 8 cores' [128] partials.
"""

import numpy as np

import concourse.bass as bass
import concourse.bacc as bacc
import concourse.bass_utils as bass_utils
import concourse.mybir as mybir
from concourse.tile import TileContext

B, W, N, D = 32, 256, 4096, 3
N_CORES = 8
BPC = B // N_CORES          # batches per core = 4
WCHUNKS = W // 128          # waypoint chunks of 128 per batch
HALF = N // 2               # 2048 columns per PSUM half
QUART = N // 4              # folded-twice length (1024)
TILES = BPC * WCHUNKS       # 8 (batch, wchunk) tiles per core

F32 = mybir.dt.float32
I32 = mybir.dt.int32
U32 = mybir.dt.uint32
ALU = mybir.AluOpType
ACTF = mybir.ActivationFunctionType


def build_bass():
    nc = bacc.Bacc()

    # ---- DRAM I/O (host-packed layouts; see make_in_maps) ----
    # lhsT source [6, BPC*W]: rows 0..2 wp^T per batch, rows 3..5 = -0.5
    wpTa = nc.dram_tensor("wpTa", [6, BPC * W], F32, kind="ExternalInput")
    # rhs source [6, BPC*N]: rows 0..2 bp^T, rows 3..5 bp^T squared
    rba = nc.dram_tensor("rba", [6, BPC * N], F32, kind="ExternalInput")
    # waypoints as [128 partitions, TILES, 3]
    wpb = nc.dram_tensor("wpb", [128, TILES * D], F32, kind="ExternalInput")
    # candidate table [BPC*QUART, 24]: row (b*QUART+j) holds bp/nrm for
    # the 4 aliases {j, j+1024, j+2048, j+3072} in ascending-index order
    gsrc = nc.dram_tensor("gsrc", [BPC * QUART, 8 * D], F32,
                          kind="ExternalInput")
    res = nc.dram_tensor("res", [128, 1], F32, kind="ExternalOutput")

    with TileContext(nc) as tc:
        with (
            tc.tile_pool(name="const", bufs=1) as cpool,
            tc.tile_pool(name="big", bufs=1) as bigpool,
            tc.tile_pool(name="work", bufs=3) as wpool,
            tc.tile_pool(name="small", bufs=6) as spool,
            tc.tile_pool(name="psum", bufs=2, space="PSUM") as psumpool,
        ):
            # ---- prep ----
            wa = cpool.tile([6, BPC * W], F32)
            nc.sync.dma_start(out=wa[:], in_=wpTa[:])
            rb_bs = []
            for b in range(BPC):
                rb_b = bigpool.tile([6, N], F32, tag=f"rb{b}")
                nc.sync.dma_start(out=rb_b[:], in_=rba[:, b * N:(b + 1) * N])
                rb_bs.append(rb_b)
            wp_all = cpool.tile([128, TILES, D], F32)
            nc.sync.dma_start(out=wp_all[:], in_=wpb[:].rearrange(
                "p (t d) -> p t d", d=D))

            gall = cpool.tile([128, TILES, 8 * D], F32)
            dots = cpool.tile([128, TILES], F32)

            # ---- PE warm-up matmuls: pre-observe prep semaphores so hot
            # matmuls carry few waits ----
            warm = psumpool.tile([128, HALF], F32, tag="score")
            nc.tensor.matmul(out=warm[0:1, 0:1], lhsT=wa[:, 0:1],
                             rhs=wa[:, 1:2], start=True, stop=True)
            for k in range(BPC):
                nc.tensor.matmul(out=warm[0:1, k + 1:k + 2], lhsT=wa[:, 0:1],
                                 rhs=rb_bs[k][:, 0:1], start=True, stop=True)

            def verify(t0, t1):
                """Pick the true nearest of the 4 candidates and write
                dot(delta, normal) into dots[:, t0:t1]."""
                n = t1 - t0
                ds, dots_c = [], []
                for ci in range(4):
                    bpC = gall[:, t0:t1, 2 * D * ci:2 * D * ci + D]
                    nrC = gall[:, t0:t1, 2 * D * ci + D:2 * D * ci + 2 * D]
                    sub = cpool.tile([128, n, D], F32, tag=f"sub{ci}_{t0}",
                                     name=f"sub{ci}_{t0}")
                    nc.vector.tensor_tensor(out=sub[:],
                                            in0=wp_all[:, t0:t1, :],
                                            in1=bpC, op=ALU.subtract)
                    sq = cpool.tile([128, n, D], F32, tag=f"sq{ci}_{t0}",
                                    name=f"sq{ci}_{t0}")
                    nc.vector.tensor_tensor(out=sq[:], in0=sub[:], in1=sub[:],
                                            op=ALU.mult)
                    dc = cpool.tile([128, n], F32, tag=f"d{ci}_{t0}",
                                    name=f"d{ci}_{t0}")
                    nc.vector.reduce_sum(out=dc[:], in_=sq[:],
                                         axis=mybir.AxisListType.X)
                    p = cpool.tile([128, n, D], F32, tag=f"p{ci}_{t0}",
                                   name=f"p{ci}_{t0}")
                    nc.vector.tensor_tensor(out=p[:], in0=sub[:], in1=nrC,
                                            op=ALU.mult)
                    dt = cpool.tile([128, n], F32, tag=f"dt{ci}_{t0}",
                                    name=f"dt{ci}_{t0}")
                    nc.vector.reduce_sum(out=dt[:], in_=p[:],
                                         axis=mybir.AxisListType.X)
                    ds.append(dc)
                    dots_c.append(dt)
                # pairwise min-tree preferring the lower index on ties
                m01 = cpool.tile([128, n], U32, tag=f"m01_{t0}",
                                 name=f"m01_{t0}")
                nc.vector.tensor_tensor(out=m01[:], in0=ds[0][:],
                                        in1=ds[1][:], op=ALU.is_le)
                m23 = cpool.tile([128, n], U32, tag=f"m23_{t0}",
                                 name=f"m23_{t0}")
                nc.vector.tensor_tensor(out=m23[:], in0=ds[2][:],
                                        in1=ds[3][:], op=ALU.is_le)
                d01 = cpool.tile([128, n], F32, tag=f"d01_{t0}",
                                 name=f"d01_{t0}")
                nc.vector.tensor_tensor(out=d01[:], in0=ds[0][:],
                                        in1=ds[1][:], op=ALU.min)
                d23 = cpool.tile([128, n], F32, tag=f"d23_{t0}",
                                 name=f"d23_{t0}")
                nc.vector.tensor_tensor(out=d23[:], in0=ds[2][:],
                                        in1=ds[3][:], op=ALU.min)
                mf = cpool.tile([128, n], U32, tag=f"mf_{t0}",
                                name=f"mf_{t0}")
                nc.vector.tensor_tensor(out=mf[:], in0=d01[:], in1=d23[:],
                                        op=ALU.is_le)
                dot01 = cpool.tile([128, n], F32, tag=f"dot01_{t0}",
                                   name=f"dot01_{t0}")
                nc.vector.tensor_copy(dot01[:], dots_c[1][:])
                nc.vector.copy_predicated(dot01[:], m01[:], dots_c[0][:])
                dot23 = cpool.tile([128, n], F32, tag=f"dot23_{t0}",
                                   name=f"dot23_{t0}")
                nc.vector.tensor_copy(dot23[:], dots_c[3][:])
                nc.vector.copy_predicated(dot23[:], m23[:], dots_c[2][:])
                nc.vector.tensor_copy(dots[:, t0:t1], dot23[:])
                nc.vector.copy_predicated(dots[:, t0:t1], mf[:], dot01[:])

            # ---- main loop ----
            for t in range(TILES):
                b, wc = divmod(t, WCHUNKS)
                lhsT = wa[:, b * W + 128 * wc:b * W + 128 * (wc + 1)]
                h0sb = wpool.tile([128, HALF], F32, tag="h0sb")
                folded = wpool.tile([128, HALF], F32, tag="folded")
                for h in range(2):
                    score = psumpool.tile([128, HALF], F32, tag="score")
                    for i in range(HALF // 512):
                        col0 = h * HALF + i * 512
                        nc.tensor.matmul(
                            out=score[:, i * 512:(i + 1) * 512],
                            lhsT=lhsT,
                            rhs=rb_bs[b][:, col0:col0 + 512],
                            start=True, stop=True)
                    if h == 0:
                        nc.scalar.copy(out=h0sb[:], in_=score[:])
                    else:
                        nc.vector.tensor_tensor(
                            out=folded[:], in0=score[:], in1=h0sb[:],
                            op=ALU.max)
                f2 = wpool.tile([128, QUART], F32, tag="f2")
                nc.vector.tensor_tensor(out=f2[:], in0=folded[:, :QUART],
                                        in1=folded[:, QUART:], op=ALU.max)
                v8 = spool.tile([128, 8], F32, tag="v8", bufs=9)
                nc.vector.max(out=v8[:], in_=f2[:])
                i8 = spool.tile([128, 8], U32, tag="i8", bufs=9)
                nc.vector.max_index(out=i8[:], in_max=v8[:],
                                    in_values=f2[:])
                idxf = spool.tile([128, 1], F32, tag="idxf", bufs=9)
                nc.vector.tensor_scalar(
                    out=idxf[:], in0=i8[:, 0:1], scalar1=float(b * QUART),
                    scalar2=None, op0=ALU.add)
                idxi = spool.tile([128, 1], I32, tag="idxi", bufs=9)
                nc.vector.tensor_copy(idxi[:], idxf[:])
                nc.gpsimd.indirect_dma_start(
                    out=gall[:, t, :], out_offset=None, in_=gsrc[:],
                    in_offset=bass.IndirectOffsetOnAxis(
                        ap=idxi[:, :1], axis=0))

            verify(0, TILES)

            # ---- exp_relu + reduction tail ----
            e = cpool.tile([128, TILES], F32)
            nc.scalar.activation(out=e[:], in_=dots[:], func=ACTF.Exp,
                                 scale=0.5)
            em1 = cpool.tile([128, TILES], F32)
            nc.vector.tensor_scalar(out=em1[:], in0=e[:], scalar1=-1.0,
                                    scalar2=None, op0=ALU.add)
            gmask = cpool.tile([128, TILES], U32)
            nc.vector.tensor_scalar(out=gmask[:], in0=dots[:], scalar1=0.0,
                                    scalar2=None, op0=ALU.is_gt)
            nc.vector.copy_predicated(em1[:], gmask[:], dots[:])
            sums = cpool.tile([128, 1], F32)
            nc.vector.reduce_sum(out=sums[:], in_=em1[:],
                                 axis=mybir.AxisListType.X)
            nc.sync.dma_start(out=res[:], in_=sums[:])

    nc.finalize()
    return nc


_NC_CACHE = None


def _get_nc():
    global _NC_CACHE
    if _NC_CACHE is None:
        _NC_CACHE = build_bass()
    return _NC_CACHE


def make_in_maps(waypoints, boundarypoints, boundarynormals):
    waypoints = np.ascontiguousarray(waypoints, dtype=np.float32)
    boundarypoints = np.ascontiguousarray(boundarypoints, dtype=np.float32)
    boundarynormals = np.ascontiguousarray(boundarynormals, dtype=np.float32)
    in_maps = []
    for c in range(N_CORES):
        sl = slice(c * BPC, (c + 1) * BPC)
        wp_c = waypoints[sl]                      # [4, 256, 3]
        bp_c = boundarypoints[sl]                 # [4, 4096, 3]
        nrm_c = boundarynormals[sl]               # [4, 4096, 3]
        wpTa = np.full((6, BPC * W), -0.5, dtype=np.float32)
        wpTa[0:3, :] = wp_c.transpose(2, 0, 1).reshape(D, BPC * W)
        bpTr = bp_c.transpose(2, 0, 1).reshape(D, BPC * N)
        rba = np.concatenate([bpTr, bpTr * bpTr], axis=0)
        wpb = np.empty((128, TILES, D), dtype=np.float32)
        for t in range(TILES):
            b, wc = divmod(t, WCHUNKS)
            wpb[:, t, :] = wp_c[b, 128 * wc:128 * (wc + 1), :]
        gsrc = np.concatenate(
            [bp_c[:, 0 * QUART:1 * QUART], nrm_c[:, 0 * QUART:1 * QUART],
             bp_c[:, 1 * QUART:2 * QUART], nrm_c[:, 1 * QUART:2 * QUART],
             bp_c[:, 2 * QUART:3 * QUART], nrm_c[:, 2 * QUART:3 * QUART],
             bp_c[:, 3 * QUART:4 * QUART], nrm_c[:, 3 * QUART:4 * QUART]],
            axis=2).reshape(BPC * QUART, 8 * D)
        in_maps.append({
            "wpTa": wpTa,
            "rba": np.ascontiguousarray(rba),
            "wpb": np.ascontiguousarray(wpb.reshape(128, TILES * D)),
            "gsrc": np.ascontiguousarray(gsrc),
        })
    return in_maps


def run_on_device(waypoints, boundarypoints, boundarynormals, trace=False):
    nc = _get_nc()
    in_maps = make_in_maps(waypoints, boundarypoints, boundarynormals)
    out = bass_utils.run_bass_kernel_spmd(
        nc, in_maps, core_ids=list(range(N_CORES)), trace=trace)
    total = np.float64(0.0)
    for r in out.results:
        total += np.sum(r["res"], dtype=np.float64)
    value = np.float32(total / (B * W))
    return value, out


def kernel(waypoints, boundarypoints, boundarynormals):
    value, _ = run_on_device(waypoints, boundarypoints, boundarynormals)
    return np.asarray(value, dtype=np.float32)



# revision 16
# speedup vs baseline: 1.3791x; 1.3791x over previous
"""BoundaryLoss kernel for Trainium2 (8 NeuronCores, data-parallel over batch).

Problem: for each (batch, waypoint), find the nearest boundary point (argmin
over N=4096 of euclidean distance), take dot(waypoint - closest_pt,
closest_normal), apply exp_relu, and mean over everything.

Key structure (per core: 4 batches; per batch 2 chunks of 128 waypoints):
  - Scores s[w, n] = w.b_n - 0.5||b_n||^2 (argmax_n s == argmin_n dist) are
    computed with float32r matmuls (1 PE cycle/row vs 4 for fp32). f32r
    rounds inputs to 12 mantissa bits, so every operand is Dekker-split into
    hi+lo halves host-side; the K dim grows 6->15 rows and the products
    reconstruct full fp32 scores exactly (PE cost depends only on moving
    columns, so this is free).
  - Level-1 fold (4096->2048) runs on PE+ACT instead of DVE/Pool:
    max(s0,s1) = s0 + relu(s1-s0). One matmul computes diff = s0-s1 from
    host-precomputed difference columns, ACT applies relu(-diff), and an
    identity matmul accumulates the relu back into s0's PSUM banks. The
    identity matmul re-rounds the relu values to 12 bits, so boundary points
    are Morton-ordered host-side and paired (q, q+2048) with near neighbors,
    keeping |s1-s0| small and the rounding harmless (~4e-3 rel on the loss).
  - Levels 2-3 fold 2048->512 on DVE/Pool, then max8 + max_index give the
    fold position p; the 8 aliases {p+512k} are fetched with one indirect
    DMA per tile from a per-batch table and re-scored exactly with vector
    ops (is_ge tree prefers the lowest alias on ties).
  - exp_relu + row-sum on device; host sums the 8 cores' [128] partials.
"""

import numpy as np

import concourse.bass as bass
import concourse.bacc as bacc
import concourse.bass_utils as bass_utils
import concourse.mybir as mybir
from concourse.tile import TileContext

B, W, N, D = 32, 256, 4096, 3
N_CORES = 8
BPC = B // N_CORES          # batches per core = 4
WCHUNKS = W // 128          # waypoint chunks of 128 per batch
TILES = BPC * WCHUNKS       # 8 (batch, wchunk) tiles per core
HALF = N // 2               # 2048: level-1 fold output length
QUART = N // 4              # 1024: level-2 output
L = N // 8                  # 512: folded length fed to max8
NAL = 8                     # aliases per fold position
ROWF = 8 * NAL              # floats per gather-table row (8 per alias)

TRICK = True                # level-1 fold on PE+ACT (False: fold on DVE/Pool)

F32 = mybir.dt.float32
F32R = mybir.dt.float32r
I32 = mybir.dt.int32
U32 = mybir.dt.uint32
ALU = mybir.AluOpType
ACTF = mybir.ActivationFunctionType
AX = mybir.AxisListType


def build_bass():
    nc = bacc.Bacc()

    # ---- DRAM I/O (host-packed; see make_in_maps) ----
    wa = nc.dram_tensor("wa", [16, BPC * W], F32R, kind="ExternalInput")
    rb = nc.dram_tensor("rb", [16, BPC * N], F32R, kind="ExternalInput")
    rbd = nc.dram_tensor("rbd", [16, BPC * HALF], F32R, kind="ExternalInput")
    wpb = nc.dram_tensor("wpb", [128, TILES * D], F32, kind="ExternalInput")
    idn = nc.dram_tensor("idn", [128, 128], F32R, kind="ExternalInput")
    gsrcs = [nc.dram_tensor(f"gsrc{b}", [L, ROWF], F32, kind="ExternalInput")
             for b in range(BPC)]
    res = nc.dram_tensor("res", [128, 1], F32, kind="ExternalOutput")

    with TileContext(nc) as tc:
        with (
            tc.tile_pool(name="const", bufs=1) as cpool,
            tc.tile_pool(name="big", bufs=1) as bigpool,
            tc.tile_pool(name="work", bufs=2) as wpool,
            tc.tile_pool(name="small", bufs=4) as spool,
            tc.tile_pool(name="psum", bufs=1, space="PSUM") as psumpool,
        ):
            # ---- input loads ----
            wat = cpool.tile([16, BPC * W], F32R)
            nc.sync.dma_start(out=wat[:], in_=wa[:])
            idt = cpool.tile([128, 128], F32R)
            nc.sync.dma_start(out=idt[:], in_=idn[:])
            rb_bs, rbd_bs = [], []
            for b in range(BPC):
                rb_b = bigpool.tile([16, N], F32R, tag=f"rb{b}")
                nc.sync.dma_start(out=rb_b[:], in_=rb[:, b * N:(b + 1) * N])
                rb_bs.append(rb_b)
                rbd_b = bigpool.tile([16, HALF], F32R, tag=f"rbd{b}")
                nc.sync.dma_start(out=rbd_b[:],
                                  in_=rbd[:, b * HALF:(b + 1) * HALF])
                rbd_bs.append(rbd_b)
            wp_all = cpool.tile([128, TILES, D], F32)
            nc.sync.dma_start(out=wp_all[:], in_=wpb[:].rearrange(
                "p (t d) -> p t d", d=D))

            gall = cpool.tile([128, TILES, ROWF], F32)
            dots = cpool.tile([128, TILES], F32)

            # PSUM: Y holds s0 then (with TRICK) max(s0,s1); X holds diff.
            Y = psumpool.tile([128, HALF], F32, tag="Y")
            X = psumpool.tile([128, HALF], F32, tag="X")

            # ---- PE warm-up: pre-observe prep semaphores; build the
            # p-state busy streak before the hot loop ----
            nc.tensor.matmul(out=Y[0:1, 0:2], lhsT=wat[:, 0:1],
                             rhs=wat[:, 0:2], start=True, stop=True)
            nc.tensor.matmul(out=Y[0:1, 2:4], lhsT=idt[:, 0:1],
                             rhs=idt[:, 0:2], start=True, stop=True)
            for k in range(BPC):
                nc.tensor.matmul(out=Y[0:1, 4 * k + 4:4 * k + 6],
                                 lhsT=wat[:, 0:1],
                                 rhs=rb_bs[k][:, 0:2], start=True, stop=True)
                nc.tensor.matmul(out=Y[0:1, 4 * k + 6:4 * k + 8],
                                 lhsT=wat[:, 0:1],
                                 rhs=rbd_bs[k][:, 0:2], start=True, stop=True)

            i8s = []

            # ---- main loop ----
            for t in range(TILES):
                b, wc = divmod(t, WCHUNKS)
                lhsT = wat[:, b * W + 128 * wc:b * W + 128 * (wc + 1)]

                if TRICK:
                    a = wpool.tile([128, HALF], F32R, tag="a")
                    # diff = s0 - s1 into X, bank by bank; ACT relus each
                    # bank into a; s0 into Y; identity matmul adds a into Y.
                    for k in range(4):
                        sl = slice(512 * k, 512 * (k + 1))
                        nc.tensor.matmul(out=X[:, sl], lhsT=lhsT,
                                         rhs=rbd_bs[b][:, sl],
                                         start=True, stop=True)
                        if k % 2:
                            sl2 = slice(512 * (k - 1), 512 * (k + 1))
                            nc.scalar.activation(out=a[:, sl2], in_=X[:, sl2],
                                                 func=ACTF.Relu, scale=-1.0)
                    for k in range(4):
                        sl = slice(512 * k, 512 * (k + 1))
                        nc.tensor.matmul(out=Y[:, sl], lhsT=lhsT,
                                         rhs=rb_bs[b][:, sl],
                                         start=True, stop=False)
                    for k in range(4):
                        sl = slice(512 * k, 512 * (k + 1))
                        nc.tensor.matmul(out=Y[:, sl], lhsT=idt[:],
                                         rhs=a[:, sl],
                                         start=False, stop=True)
                    # Vector ops can read only one PSUM operand, gpsimd
                    # cannot touch PSUM, and Pool has no max op — so ACT
                    # copies the right half of m1 (= Y) to SBUF and DVE folds.
                    c = wpool.tile([128, QUART], F32, tag="c")
                    nc.scalar.copy(out=c[:], in_=Y[:, 1024:2048])
                    f2 = wpool.tile([128, QUART], F32, tag="f2")
                    nc.vector.tensor_tensor(
                        out=f2[:], in0=Y[:, 0:1024], in1=c[:], op=ALU.max)
                else:
                    for k in range(4):
                        sl = slice(512 * k, 512 * (k + 1))
                        nc.tensor.matmul(out=Y[:, sl], lhsT=lhsT,
                                         rhs=rb_bs[b][:, sl],
                                         start=True, stop=True)
                        nc.tensor.matmul(out=X[:, sl], lhsT=lhsT,
                                         rhs=rb_bs[b][:, 2048 + 512 * k:
                                                       2048 + 512 * (k + 1)],
                                         start=True, stop=True)
                    # copy X (= s1) to SBUF, then fold against PSUM Y
                    c = wpool.tile([128, HALF], F32, tag="c")
                    nc.scalar.copy(out=c[:, 0:1024], in_=X[:, 0:1024])
                    nc.scalar.copy(out=c[:, 1024:2048], in_=X[:, 1024:2048])
                    f1 = wpool.tile([128, HALF], F32, tag="f1")
                    nc.vector.tensor_tensor(
                        out=f1[:, 0:1024], in0=Y[:, 0:1024],
                        in1=c[:, 0:1024], op=ALU.max)
                    nc.vector.tensor_tensor(
                        out=f1[:, 1024:2048], in0=Y[:, 1024:2048],
                        in1=c[:, 1024:2048], op=ALU.max)
                    f2 = wpool.tile([128, QUART], F32, tag="f2")
                    nc.vector.tensor_tensor(
                        out=f2[:], in0=f1[:, 0:1024],
                        in1=f1[:, 1024:2048], op=ALU.max)

                # level 3 + top-1
                f3 = wpool.tile([128, L], F32, tag="f3")
                nc.vector.tensor_tensor(out=f3[:], in0=f2[:, 0:512],
                                        in1=f2[:, 512:1024], op=ALU.max)
                v8 = spool.tile([128, 8], F32, tag="v8", bufs=4)
                nc.vector.max(out=v8[:], in_=f3[:])
                i8 = spool.tile([128, 8], U32, tag="i8", bufs=4,
                                name=f"i8_{t}")
                nc.vector.max_index(out=i8[:], in_max=v8[:], in_values=f3[:])
                i8s.append(i8)
                nc.gpsimd.indirect_dma_start(
                    out=gall[:, t, :], out_offset=None, in_=gsrcs[b][:],
                    in_offset=bass.IndirectOffsetOnAxis(
                        ap=i8[:, 0:1].bitcast(I32), axis=0))

            # ---- verify: among the 8 aliases pick the true nearest and
            # emit dot(w - b, n); gall row per alias: [bx by bz hb nx ny nz c]
            def verify(t0, t1):
                n = t1 - t0
                g = gall[:, t0:t1, :].rearrange("p t (a f) -> p t a f", f=8)
                wpv = wp_all[:, t0:t1, :].unsqueeze(2).broadcast_to(
                    [128, n, NAL, D])
                pr = cpool.tile([128, n, NAL, D], F32, tag=f"pr{t0}",
                                name=f"pr{t0}")
                nc.vector.tensor_tensor(out=pr[:], in0=wpv,
                                        in1=g[:, :, :, 0:3], op=ALU.mult)
                wb = cpool.tile([128, n, NAL], F32, tag=f"wb{t0}",
                                name=f"wb{t0}")
                nc.vector.tensor_reduce(out=wb[:], in_=pr[:], axis=AX.X,
                                        op=ALU.add)
                pr2 = cpool.tile([128, n, NAL, D], F32, tag=f"pr2{t0}",
                                 name=f"pr2{t0}")
                nc.vector.tensor_tensor(out=pr2[:], in0=wpv,
                                        in1=g[:, :, :, 4:7], op=ALU.mult)
                wn = cpool.tile([128, n, NAL], F32, tag=f"wn{t0}",
                                name=f"wn{t0}")
                nc.vector.tensor_reduce(out=wn[:], in_=pr2[:], axis=AX.X,
                                        op=ALU.add)
                sc = cpool.tile([128, n, NAL], F32, tag=f"sc{t0}",
                                name=f"sc{t0}")
                nc.gpsimd.tensor_tensor(out=sc[:], in0=wb[:],
                                        in1=g[:, :, :, 3], op=ALU.subtract)
                dt = cpool.tile([128, n, NAL], F32, tag=f"dt{t0}",
                                name=f"dt{t0}")
                nc.gpsimd.tensor_tensor(out=dt[:], in0=wn[:],
                                        in1=g[:, :, :, 7], op=ALU.subtract)
                # pairwise tournament, lower alias wins ties
                scur, dcur = sc, dt
                wdt = NAL
                while wdt > 1:
                    wdt //= 2
                    ev = (slice(None), slice(None), slice(0, 2 * wdt, 2))
                    od = (slice(None), slice(None), slice(1, 2 * wdt, 2))
                    m = cpool.tile([128, n, wdt], U32, tag=f"m{t0}_{wdt}",
                                   name=f"m{t0}_{wdt}")
                    nc.vector.tensor_tensor(out=m[:], in0=scur[ev],
                                            in1=scur[od], op=ALU.is_ge)
                    s2 = cpool.tile([128, n, wdt], F32, tag=f"s{t0}_{wdt}",
                                    name=f"s{t0}_{wdt}")
                    nc.vector.tensor_tensor(out=s2[:], in0=scur[ev],
                                            in1=scur[od], op=ALU.max)
                    d2 = cpool.tile([128, n, wdt], F32, tag=f"d{t0}_{wdt}",
                                    name=f"d{t0}_{wdt}")
                    nc.vector.tensor_copy(d2[:], dcur[od])
                    nc.vector.copy_predicated(d2[:], m[:], dcur[ev])
                    scur, dcur = s2, d2
                nc.vector.tensor_copy(dots[:, t0:t1], dcur[:, :, 0])

            verify(0, 6)
            verify(6, 8)

            # ---- exp_relu + reduction tail ----
            e = cpool.tile([128, TILES], F32)
            nc.scalar.activation(out=e[:], in_=dots[:], func=ACTF.Exp,
                                 scale=0.5)
            em1 = cpool.tile([128, TILES], F32)
            nc.vector.tensor_scalar(out=em1[:], in0=e[:], scalar1=-1.0,
                                    scalar2=None, op0=ALU.add)
            gmask = cpool.tile([128, TILES], U32)
            nc.vector.tensor_scalar(out=gmask[:], in0=dots[:], scalar1=0.0,
                                    scalar2=None, op0=ALU.is_gt)
            nc.vector.copy_predicated(em1[:], gmask[:], dots[:])
            sums = cpool.tile([128, 1], F32)
            nc.vector.reduce_sum(out=sums[:], in_=em1[:], axis=AX.X)
            nc.sync.dma_start(out=res[:], in_=sums[:])

    nc.finalize()
    return nc


_NC_CACHE = None


def _get_nc():
    global _NC_CACHE
    if _NC_CACHE is None:
        _NC_CACHE = build_bass()
    return _NC_CACHE


def _split12(x):
    """Split fp32 array into hi (top 12 mantissa bits, f32r-exact) + lo."""
    x = np.asarray(x, dtype=np.float32)
    c = np.float32((1 << 12) + 1)
    t = (c * x).astype(np.float32)
    hi = (t - (t - x).astype(np.float32)).astype(np.float32)
    lo = (x - hi).astype(np.float32)
    return hi, lo


def _morton_perm(bp):
    """Order boundary points so consecutive points are spatial neighbors,
    then lay pairs out as (q, q+HALF)."""
    lo = bp.min(0)
    span = bp.max(0) - lo + 1e-9
    q = np.floor((bp - lo) / span * 31.999).astype(np.int64)
    code = np.zeros(bp.shape[0], dtype=np.int64)
    for i in range(5):
        for d in range(3):
            code |= ((q[:, d] >> i) & 1) << (3 * i + d)
    order = np.argsort(code, kind="stable")
    perm = np.empty(bp.shape[0], dtype=np.int64)
    perm[:HALF] = order[0::2]
    perm[HALF:] = order[1::2]
    return perm


def _pack_rhs16(bpP):
    """[16, N] f32 block for the score matmul from permuted bp [N, 3]."""
    bT = bpP.T.astype(np.float32)                      # [3, N]
    sq = (bT * bT).astype(np.float32)                  # [3, N]
    bh, bl = _split12(bT)
    sh, sl = _split12(sq)
    out = np.zeros((16, bpP.shape[0]), dtype=np.float32)
    out[0:3] = bh
    out[3:6] = bl
    out[6:9] = bh
    out[9:12] = sh
    out[12:15] = sl
    return out


def make_in_maps(waypoints, boundarypoints, boundarynormals):
    waypoints = np.ascontiguousarray(waypoints, dtype=np.float32)
    boundarypoints = np.ascontiguousarray(boundarypoints, dtype=np.float32)
    boundarynormals = np.ascontiguousarray(boundarynormals, dtype=np.float32)
    in_maps = []
    for c in range(N_CORES):
        sl = slice(c * BPC, (c + 1) * BPC)
        wp_c = waypoints[sl]                      # [4, 256, 3]
        bp_c = boundarypoints[sl]                 # [4, 4096, 3]
        nrm_c = boundarynormals[sl]               # [4, 4096, 3]

        # lhsT [16, BPC*W]: rows wh, wh, wl, -0.5 x6, 0
        wT = wp_c.transpose(0, 2, 1).reshape(BPC, D, W)
        wa = np.zeros((16, BPC * W), dtype=np.float32)
        wa[9:15] = -0.5
        for b in range(BPC):
            wh, wl = _split12(wT[b])
            blk = slice(b * W, (b + 1) * W)
            wa[0:3, blk] = wh
            wa[3:6, blk] = wh
            wa[6:9, blk] = wl

        rb = np.zeros((16, BPC * N), dtype=np.float32)
        rbd = np.zeros((16, BPC * HALF), dtype=np.float32)
        gsrcs = {}
        for b in range(BPC):
            perm = _morton_perm(bp_c[b])
            bpP = bp_c[b][perm]                   # [N, 3] permuted
            nrP = nrm_c[b][perm]
            rb[:, b * N:(b + 1) * N] = _pack_rhs16(bpP)
            # difference columns for the level-1 trick: col q - col q+HALF
            db = (bpP[:HALF] - bpP[HALF:]).astype(np.float32)
            sqP = (bpP * bpP).astype(np.float32)
            ds = (sqP[:HALF] - sqP[HALF:]).astype(np.float32)
            dbh, dbl = _split12(db.T)
            dsh, dsl = _split12(ds.T)
            blk = slice(b * HALF, (b + 1) * HALF)
            rbd[0:3, blk] = dbh
            rbd[3:6, blk] = dbl
            rbd[6:9, blk] = dbh
            rbd[9:12, blk] = dsh
            rbd[12:15, blk] = dsl
            # gather table: row p, alias k at perm position p + L*k:
            # [bx by bz hb nx ny nz c]
            g = np.empty((L, ROWF), dtype=np.float32)
            for k in range(NAL):
                seg = slice(p0 := L * k, p0 + L)
                bseg = bpP[seg]
                nseg = nrP[seg]
                g[:, 8 * k:8 * k + 3] = bseg
                g[:, 8 * k + 3] = 0.5 * (bseg * bseg).sum(1, dtype=np.float32)
                g[:, 8 * k + 4:8 * k + 7] = nseg
                g[:, 8 * k + 7] = (bseg * nseg).sum(1, dtype=np.float32)
            gsrcs[f"gsrc{b}"] = np.ascontiguousarray(g)

        wpb = np.empty((128, TILES, D), dtype=np.float32)
        for t in range(TILES):
            b, wc = divmod(t, WCHUNKS)
            wpb[:, t, :] = wp_c[b, 128 * wc:128 * (wc + 1), :]

        in_maps.append({
            "wa": wa,
            "rb": np.ascontiguousarray(rb),
            "rbd": np.ascontiguousarray(rbd),
            "wpb": np.ascontiguousarray(wpb.reshape(128, TILES * D)),
            "idn": np.ascontiguousarray(np.eye(128, dtype=np.float32)),
            **gsrcs,
        })
    return in_maps


def run_on_device(waypoints, boundarypoints, boundarynormals, trace=False):
    nc = _get_nc()
    in_maps = make_in_maps(waypoints, boundarypoints, boundarynormals)
    out = bass_utils.run_bass_kernel_spmd(
        nc, in_maps, core_ids=list(range(N_CORES)), trace=trace)
    total = np.float64(0.0)
    for r in out.results:
        total += np.sum(r["res"], dtype=np.float64)
    value = np.float32(total / (B * W))
    return value, out


def kernel(waypoints, boundarypoints, boundarynormals):
    value, _ = run_on_device(waypoints, boundarypoints, boundarynormals)
    return np.asarray(value, dtype=np.float32)
